# revision 3
# baseline (speedup 1.0000x reference)
"""Trainium2 Bass kernel for circular 3x3 conv (im2col-free shift-pair GEMM).

out[b,h,w,f] = sum_{dh,dw,c} x[b,(h-dh)%H,(w-dw)%W,c] * K[j*C+c, f] + bias[f]
with j = dw_idx*3 + dh_idx, dh = [-1,0,1][dh_idx], dw = [-1,0,1][dw_idx].

Per core (8 cores, 2 batches each):
  - x loaded partition=h, free=(w,c), in 8 w-blocks of 17 cols (1 overlap) ->
    4.3KB contiguous DMA descriptors (line rate).
  - PE transpose of adjacent w-col pairs -> slab S[w] = [c(x[:,w]) ; c(x[:,w+1])]
    on 128 partitions, free = h (+2 circular pad cols), rounded to fp32r by the
    DVE PSUM->SBUF copy.
  - Per output col w: 3 pair matmuls (K=128, slab[w], shifts (dw=0,-1) x dh)
    + 3 single matmuls (K=64, slab[w-1] top half, dw=+1) accumulate into
    PSUM [128 h, 256 f]; DVE adds bias -> SBUF; 1KB-strided DMA to DRAM.
  - Slab production software-pipelined 2 steps ahead of consumption.
"""
import numpy as np

B, H, W, C, F = 16, 128, 128, 64, 256
NCORES = 8
BPC = B // NCORES  # batches per core
NBLK = 8  # w blocks per batch
BLKW = W // NBLK  # 16 cols per block (tiles hold 17, one overlap col)


def _build_module():
    import concourse.bacc as bacc
    import concourse.mybir as mybir
    import concourse.tile as tile

    f32 = mybir.dt.float32
    f32r = mybir.dt.float32r

    nc = bacc.Bacc("TRN2", target_bir_lowering=False, debug=False,
                   num_devices=NCORES)
    xc_d = nc.dram_tensor("xc", [BPC, H, W, C], f32, kind="ExternalInput").ap()
    kw_d = nc.dram_tensor("kw", [9 * C, F], f32, kind="ExternalInput").ap()
    biasf_d = nc.dram_tensor("biasf", [128, F], f32, kind="ExternalInput").ap()
    ident_d = nc.dram_tensor("ident", [128, 128], f32, kind="ExternalInput").ap()
    out_d = nc.dram_tensor("out", [BPC, H, W, F], f32, kind="ExternalOutput").ap()

    with tile.TileContext(nc) as tc:
        with (
            tc.tile_pool(name="persist", bufs=1) as persist,
            tc.tile_pool(name="kraw", bufs=2) as kraw_pool,
            tc.tile_pool(name="slab_sb", bufs=10) as slab_pool,
            tc.tile_pool(name="out_sb", bufs=4) as out_pool,
            tc.tile_pool(name="ps_slab", bufs=4, space="PSUM") as ps_slab,
            tc.tile_pool(name="ps_out", bufs=4, space="PSUM") as ps_out,
        ):
            # ---- static prep: identity, bias, kernel tiles (fp32r) ----
            ident = persist.tile([128, 128], f32, tag="ident")
            nc.sync.dma_start(ident[:], ident_d[:])
            biasf = persist.tile([128, F], f32, tag="biasf")
            nc.sync.dma_start(biasf[:], biasf_d[:])

            k_pair = []
            k_single = []
            for dhi in range(3):
                kp_raw = kraw_pool.tile([128, F], f32, tag="kpraw")
                # rows 0..63 = shift (dw=0,dh): j = 3+dhi
                nc.sync.dma_start(kp_raw[0:C, :],
                                  kw_d[(3 + dhi) * C:(4 + dhi) * C, :])
                # rows 64..127 = shift (dw=-1,dh): j = dhi
                nc.sync.dma_start(kp_raw[C:2 * C, :],
                                  kw_d[dhi * C:(dhi + 1) * C, :])
                kp = persist.tile([128, F], f32r, tag=f"kp{dhi}")
                nc.vector.tensor_copy(kp[:], kp_raw[:])
                k_pair.append(kp)

                ks_raw = kraw_pool.tile([C, F], f32, tag="ksraw")
                # shift (dw=+1,dh): j = 6+dhi
                nc.sync.dma_start(ks_raw[:],
                                  kw_d[(6 + dhi) * C:(7 + dhi) * C, :])
                ks = persist.tile([C, F], f32r, tag=f"ks{dhi}")
                nc.vector.tensor_copy(ks[:], ks_raw[:])
                k_single.append(ks)

            # ---- x block loads: [h=128, 17 w-cols, c] per (b, g) ----
            xb_t = [[None] * NBLK for _ in range(BPC)]
            for b in range(BPC):
                # block 7 first (wrap slab S[127] is produced first)
                for g in [NBLK - 1] + list(range(NBLK - 1)):
                    t = persist.tile([H, BLKW + 1, C], f32, tag=f"xb{b}_{g}")
                    ncols = BLKW + 1 if g < NBLK - 1 else BLKW
                    nc.sync.dma_start(
                        t[:, 0:ncols, :],
                        xc_d[b, :, g * BLKW:g * BLKW + ncols, :],
                    )
                    if g == NBLK - 1:  # wrap col: x[b, :, 0] into slot 16
                        nc.sync.dma_start(
                            t[:, BLKW:BLKW + 1, :], xc_d[b, :, 0:1, :]
                        )
                    xb_t[b][g] = t

            # ---- main loop ----
            def make_slab(b, w, tag):
                """Transpose cols (w, w+1) -> slab [128, H+2] fp32r in SBUF."""
                g, i = w // BLKW, w % BLKW
                ps = ps_slab.tile([128, H], f32, tag="pslab")
                nc.tensor.matmul(ps[:], xb_t[b][g][:, i:i + 2, :], ident[:],
                                 is_transpose=True, start=True, stop=True)
                sl = slab_pool.tile([128, H + 2], f32r, tag=tag)
                nc.vector.tensor_copy(sl[:, 1:H + 1], ps[:])
                nc.vector.tensor_copy(sl[:, 0:1], ps[:, H - 1:H])
                nc.vector.tensor_copy(sl[:, H + 1:H + 2], ps[:, 0:1])
                return sl

            for b in range(BPC):
                slabs = {W - 1: make_slab(b, W - 1, tag="wrap"),
                         0: make_slab(b, 0, tag="slab"),
                         1: make_slab(b, 1, tag="slab")}
                for w in range(W):
                    sl = slabs[w]
                    prev = slabs[(w - 1) % W]
                    po = ps_out.tile([H, F], f32, tag="pout")
                    for dhi in range(3):
                        off = 2 - dhi  # dh = [-1,0,1][dhi] -> off = 1-dh
                        nc.tensor.matmul(
                            po[:], sl[:, off:off + H], k_pair[dhi][:],
                            start=(dhi == 0), stop=False,
                        )
                        nc.tensor.matmul(
                            po[:], prev[0:C, off:off + H], k_single[dhi][:],
                            start=False, stop=(dhi == 2),
                        )
                    ob = out_pool.tile([H, F], f32, tag="outsb")
                    nc.vector.tensor_add(ob[:], po[:], biasf[:])
                    eng = nc.sync if w % 2 == 0 else nc.scalar
                    eng.dma_start(out_d[b, :, w, :], ob[:])
                    # software pipeline: produce slab w+2
                    if w + 2 <= W - 2:
                        slabs[w + 2] = make_slab(b, w + 2, tag="slab")
                    if w >= 1:
                        slabs.pop(w - 1, None)

    nc.compile()
    return nc


_NC_CACHE = None


def _get_nc():
    global _NC_CACHE
    if _NC_CACHE is None:
        _NC_CACHE = _build_module()
    return _NC_CACHE


def kernel(x, kernel, bias, _trace=False):
    from concourse.bass_utils import run_bass_kernel_spmd

    x = np.ascontiguousarray(np.asarray(x, dtype=np.float32))
    kern = np.ascontiguousarray(np.asarray(kernel, dtype=np.float32))
    bias = np.asarray(bias, dtype=np.float32)
    biasf = np.ascontiguousarray(np.broadcast_to(bias[None, :], (128, F)))
    ident = np.eye(128, dtype=np.float32)

    nc = _get_nc()
    in_maps = [
        {"xc": x[c * BPC:(c + 1) * BPC], "kw": kern, "biasf": biasf,
         "ident": ident}
        for c in range(NCORES)
    ]
    res = run_bass_kernel_spmd(nc, in_maps, core_ids=list(range(NCORES)),
                               trace=_trace)
    out = np.concatenate([res.results[c]["out"] for c in range(NCORES)], axis=0)
    if _trace:
        kernel._last_results = res
    return out


# revision 6
# speedup vs baseline: 2.9119x; 2.9119x over previous
"""Trainium2 Bass kernel for circular 3x3 conv (im2col-free shift-pair GEMM).

out[b,h,w,f] = sum_{dh,dw,c} x[b,(h-dh)%H,(w-dw)%W,c] * K[j*C+c, f] + bias[f]
with j = dw_idx*3 + dh_idx, dh = [-1,0,1][dh_idx], dw = [-1,0,1][dw_idx].

Per core (8 cores, 2 batches each):
  - x loaded partition=h, free=(w,c), in 8 w-blocks of 17 cols (1 overlap) ->
    4.3KB contiguous DMA descriptors (line rate).
  - PE transpose of adjacent w-col pairs -> slab S[w] = [c(x[:,w]) ; c(x[:,w+1])]
    on 128 partitions, free = h (+2 circular pad cols), rounded to fp32r by the
    DVE PSUM->SBUF copy.
  - Per output col w: 3 pair matmuls (K=128, slab[w], shifts (dw=0,-1) x dh)
    + 3 single matmuls (K=64, slab[w-1] top half, dw=+1) accumulate into
    PSUM [128 h, 256 f]; DVE adds bias -> SBUF; 1KB-strided DMA to DRAM.
  - Slab production software-pipelined 2 steps ahead of consumption.
"""
import numpy as np

B, H, W, C, F = 16, 128, 128, 64, 256
NCORES = 8
BPC = B // NCORES  # batches per core
NBLK = 8  # w blocks per batch
BLKW = W // NBLK  # 16 cols per block (tiles hold 17, one overlap col)


def _build_module(reps=1):
    import concourse.bacc as bacc
    import concourse.mybir as mybir
    import concourse.tile as tile

    f32 = mybir.dt.float32
    f32r = mybir.dt.float32r

    nc = bacc.Bacc("TRN2", target_bir_lowering=False, debug=False,
                   num_devices=NCORES)
    xc_d = nc.dram_tensor("xc", [BPC, H, W, C], f32, kind="ExternalInput").ap()
    kw_d = nc.dram_tensor("kw", [9 * C, F], f32, kind="ExternalInput").ap()
    biasf_d = nc.dram_tensor("biasf", [128, F], f32, kind="ExternalInput").ap()
    ident_d = nc.dram_tensor("ident", [128, 128], f32, kind="ExternalInput").ap()
    out_d = nc.dram_tensor("out", [BPC, H, W, F], f32, kind="ExternalOutput").ap()

    with tile.TileContext(nc) as tc:
        with (
            tc.tile_pool(name="persist", bufs=1) as persist,
            tc.tile_pool(name="kraw", bufs=2) as kraw_pool,
            tc.tile_pool(name="slab_sb", bufs=10) as slab_pool,
            tc.tile_pool(name="out_sb", bufs=4) as out_pool,
            tc.tile_pool(name="ps_slab", bufs=4, space="PSUM") as ps_slab,
            tc.tile_pool(name="ps_out", bufs=4, space="PSUM") as ps_out,
        ):
            # ---- static prep: identity, bias, kernel tiles (fp32r) ----
            ident = persist.tile([128, 128], f32, tag="ident")
            nc.sync.dma_start(ident[:], ident_d[:])
            biasf = persist.tile([128, F], f32, tag="biasf")
            nc.sync.dma_start(biasf[:], biasf_d[:])

            k_pair = []
            k_single = []
            for dhi in range(3):
                kp_raw = kraw_pool.tile([128, F], f32, tag="kpraw")
                # rows 0..63 = shift (dw=0,dh): j = 3+dhi
                nc.sync.dma_start(kp_raw[0:C, :],
                                  kw_d[(3 + dhi) * C:(4 + dhi) * C, :])
                # rows 64..127 = shift (dw=-1,dh): j = dhi
                nc.sync.dma_start(kp_raw[C:2 * C, :],
                                  kw_d[dhi * C:(dhi + 1) * C, :])
                kp = persist.tile([128, F], f32r, tag=f"kp{dhi}")
                nc.vector.tensor_copy(kp[:], kp_raw[:])
                k_pair.append(kp)

                ks_raw = kraw_pool.tile([C, F], f32, tag="ksraw")
                # shift (dw=+1,dh): j = 6+dhi
                nc.sync.dma_start(ks_raw[:],
                                  kw_d[(6 + dhi) * C:(7 + dhi) * C, :])
                ks = persist.tile([C, F], f32r, tag=f"ks{dhi}")
                nc.vector.tensor_copy(ks[:], ks_raw[:])
                k_single.append(ks)

            # ---- x block loads: [h=128, 17 w-cols, c] per (b, g) ----
            xb_t = [[None] * NBLK for _ in range(BPC)]
            for b in range(BPC):
                # block 7 first (wrap slab S[127] is produced first)
                for g in [NBLK - 1] + list(range(NBLK - 1)):
                    t = persist.tile([H, BLKW + 1, C], f32, tag=f"xb{b}_{g}")
                    ncols = BLKW + 1 if g < NBLK - 1 else BLKW
                    nc.sync.dma_start(
                        t[:, 0:ncols, :],
                        xc_d[b, :, g * BLKW:g * BLKW + ncols, :],
                    )
                    if g == NBLK - 1:  # wrap col: x[b, :, 0] into slot 16
                        nc.sync.dma_start(
                            t[:, BLKW:BLKW + 1, :], xc_d[b, :, 0:1, :]
                        )
                    xb_t[b][g] = t

            # ---- main loop ----
            def make_slab(b, w, tag):
                """Transpose cols (w, w+1) -> slab [128, H+2] fp32r in SBUF."""
                g, i = w // BLKW, w % BLKW
                ps = ps_slab.tile([128, H], f32, tag="pslab")
                nc.tensor.matmul(ps[:], xb_t[b][g][:, i:i + 2, :], ident[:],
                                 is_transpose=True, start=True, stop=True)
                sl = slab_pool.tile([128, H + 2], f32r, tag=tag)
                nc.vector.tensor_copy(sl[:, 1:H + 1], ps[:])
                nc.vector.tensor_copy(sl[:, 0:1], ps[:, H - 1:H])
                nc.vector.tensor_copy(sl[:, H + 1:H + 2], ps[:, 0:1])
                return sl

            for _rep in range(reps):
              for b in range(BPC):
                slabs = {W - 1: make_slab(b, W - 1, tag="wrap"),
                         0: make_slab(b, 0, tag="slab"),
                         1: make_slab(b, 1, tag="slab")}
                for w in range(W):
                    sl = slabs[w]
                    prev = slabs[(w - 1) % W]
                    po = ps_out.tile([H, F], f32, tag="pout")
                    for dhi in range(3):
                        off = 2 - dhi  # dh = [-1,0,1][dhi] -> off = 1-dh
                        nc.tensor.matmul(
                            po[:], sl[:, off:off + H], k_pair[dhi][:],
                            start=(dhi == 0), stop=False,
                        )
                        nc.tensor.matmul(
                            po[:], prev[0:C, off:off + H], k_single[dhi][:],
                            start=False, stop=(dhi == 2),
                        )
                    ob = out_pool.tile([H, F], f32, tag="outsb")
                    nc.vector.tensor_add(ob[:], po[:], biasf[:])
                    eng = nc.sync if w % 2 == 0 else nc.scalar
                    eng.dma_start(out_d[b, :, w, :], ob[:])
                    # software pipeline: produce slab w+2
                    if w + 2 <= W - 2:
                        slabs[w + 2] = make_slab(b, w + 2, tag="slab")
                    if w >= 1:
                        slabs.pop(w - 1, None)

    nc.compile()
    return nc


_NC_CACHE = {}


def _get_nc(reps=1):
    if reps not in _NC_CACHE:
        _NC_CACHE[reps] = _build_module(reps)
    return _NC_CACHE[reps]


def kernel(x, kernel, bias, _trace=False):
    from concourse.bass_utils import run_bass_kernel_spmd

    x = np.ascontiguousarray(np.asarray(x, dtype=np.float32))
    kern = np.ascontiguousarray(np.asarray(kernel, dtype=np.float32))
    bias = np.asarray(bias, dtype=np.float32)
    biasf = np.ascontiguousarray(np.broadcast_to(bias[None, :], (128, F)))
    ident = np.eye(128, dtype=np.float32)

    nc = _get_nc()
    in_maps = [
        {"xc": x[c * BPC:(c + 1) * BPC], "kw": kern, "biasf": biasf,
         "ident": ident}
        for c in range(NCORES)
    ]
    res = run_bass_kernel_spmd(nc, in_maps, core_ids=list(range(NCORES)),
                               trace=_trace)
    out = np.concatenate([res.results[c]["out"] for c in range(NCORES)], axis=0)
    if _trace:
        kernel._last_results = res
    return out


# revision 24
# speedup vs baseline: 199.2691x; 68.4316x over previous
"""Trainium2 Bass kernel for circular 3x3 conv (im2col-free shift-pair GEMM).

out[b,h,w,f] = sum_{dh,dw,c} x[b,(h-dh)%H,(w-dw)%W,c] * K[j*C+c, f] + bias[f]
with j = dw_idx*3 + dh_idx, dh = [-1,0,1][dh_idx], dw = [-1,0,1][dw_idx].

Per core (8 cores, 2 batches each):
  - x loaded partition=h, free=(w,c), in 8 w-blocks of 16 cols -> 4KB
    contiguous DMA descriptors (line rate).
  - PE transposes only EVEN w-col pairs -> slab E[i] = [c(x[:,2i]) ; c(x[:,2i+1])]
    on 128 partitions, free = h (+2 circular pad cols), rounded to fp32r by
    the DVE PSUM->SBUF copy.
  - Per output col w (i = w//2), 6 fp32r matmuls accumulate PSUM [128 h, 256 f]:
      even w: 3x pair K=128 from E[i] (kernel rows [dw=0; dw=-1])
              + 3x single K=64 from E[i-1] bottom half (dw=+1, array rows 64-127)
      odd w:  3x pair K=128 from E[i] (kernel rows [dw=+1; dw=0])
              + 3x single K=64 from E[i+1] top half (dw=-1)
    DVE adds bias -> SBUF; 1KB-strided DMA out. Slab production is software-
    pipelined ahead of consumption.
"""
import numpy as np

B, H, W, C, F = 16, 128, 128, 64, 256
NCORES = 8
BPC = B // NCORES  # batches per core
NBLK = 8  # w blocks per batch
BLKW = W // NBLK  # 16 cols per block
NE = W // 2  # even slabs per batch


def _build_module(reps=1):
    import concourse.bacc as bacc
    import concourse.mybir as mybir
    import concourse.tile as tile

    f32 = mybir.dt.float32
    f32r = mybir.dt.float32r

    nc = bacc.Bacc("TRN2", target_bir_lowering=False, debug=False,
                   num_devices=NCORES)
    xc_d = nc.dram_tensor("xc", [BPC, H, W, C], f32, kind="ExternalInput").ap()
    kw_d = nc.dram_tensor("kw", [9 * C, F], f32, kind="ExternalInput").ap()
    biasf_d = nc.dram_tensor("biasf", [128, F], f32, kind="ExternalInput").ap()
    ident_d = nc.dram_tensor("ident", [128, 128], f32, kind="ExternalInput").ap()
    out_d = nc.dram_tensor("out", [BPC, H, W, F], f32, kind="ExternalOutput").ap()

    with tile.TileContext(nc) as tc:
        with (
            tc.tile_pool(name="persist", bufs=1) as persist,
            tc.tile_pool(name="kraw", bufs=2) as kraw_pool,
            tc.tile_pool(name="slab_sb", bufs=7) as slab_pool,
            tc.tile_pool(name="out_sb", bufs=4) as out_pool,
            tc.tile_pool(name="ps_slab", bufs=3, space="PSUM") as ps_slab,
            tc.tile_pool(name="ps_out", bufs=5, space="PSUM") as ps_out,
        ):
            # ---- static prep: identity first (gates transposes), then
            # starters, kernel tiles, bias ----
            ident = persist.tile([128, 128], f32, tag="ident")
            nc.sync.dma_start(ident[:], ident_d[:])

            start_a = persist.tile([H, 2, C], f32, tag="start_a")  # cols 126,127
            nc.scalar.dma_start(start_a[:], xc_d[0, :, W - 2:W, :])
            start_b = persist.tile([H, 4, C], f32, tag="start_b")  # cols 0..3
            nc.scalar.dma_start(start_b[:], xc_d[0, :, 0:4, :])

            # Each tile group's top/bottom halves are contiguous 192-row
            # kernel ranges -> 2 DMAs per group, 6 total.
            # kw viewed as [9, C, F]; group tile [128, 3, F]:
            #   [0:C, dhi, :] = shift j_top+dhi, [C:2C, dhi, :] = j_bot+dhi.
            kw3 = kw_d.rearrange("(j c) f -> j c f", c=C)

            def kload3(j_top, j_bot, tag, ei):
                raw = kraw_pool.tile([128, 3, F], f32, tag=f"kraw{tag}")
                _keng = [nc.sync, nc.scalar]
                _keng[ei].dma_start(
                    raw[0:C, :, :],
                    kw3[j_top:j_top + 3, :, :].rearrange("j c f -> c j f"))
                _keng[1 - ei].dma_start(
                    raw[C:2 * C, :, :],
                    kw3[j_bot:j_bot + 3, :, :].rearrange("j c f -> c j f"))
                t = persist.tile([128, 3, F], f32r, tag=tag)
                nc.vector.tensor_copy(t[:], raw[:])
                return t

            kp1_all = kload3(3, 0, "kp1", 0)  # top j=3+dhi, bottom j=dhi
            ks_all = kload3(0, 6, "ks", 1)    # top j=dhi (lo), bottom j=6+dhi (hi)
            kp2_all = kload3(6, 3, "kp2", 0)  # top j=6+dhi, bottom j=3+dhi
            kp1 = [kp1_all[:, d, :] for d in range(3)]
            kp2 = [kp2_all[:, d, :] for d in range(3)]
            ks_lo = [ks_all[:, d, :] for d in range(3)]
            ks_hi = ks_lo

            biasf = persist.tile([128, F], f32, tag="biasf")
            nc.sync.dma_start(biasf[:], biasf_d[:])

            # ---- x loads: per batch, 3 DMAs: cols 112-127 (E63 first),
            # cols 0-15, cols 16-111 ----
            xb_t = []
            for b in range(BPC):
                t7 = persist.tile([H, 16, C], f32, tag=f"xb{b}_hi")
                nc.sync.dma_start(t7[:], xc_d[b, :, 112:128, :])
                t0 = persist.tile([H, 16, C], f32, tag=f"xb{b}_lo")
                nc.scalar.dma_start(t0[:], xc_d[b, :, 0:16, :])
                tms = []
                for ci in range(4):
                    w0 = 16 + 24 * ci
                    tm = persist.tile([H, 24, C], f32, tag=f"xb{b}_m{ci}")
                    eng = nc.sync if ci % 2 == 0 else nc.scalar
                    eng.dma_start(tm[:], xc_d[b, :, w0:w0 + 24, :])
                    tms.append(tm)
                xb_t.append((t0, tms, t7))

            # ---- main loop ----
            def make_eslab(b, i, tag, src=None, src_col=0):
                """Transpose cols (2i, 2i+1) -> slab [128, H+2] fp32r."""
                if src is None:
                    w0 = 2 * i
                    t0, tms, t7 = xb_t[b]
                    if w0 < 16:
                        src, src_col = t0, w0
                    elif w0 < 112:
                        src, src_col = tms[(w0 - 16) // 24], (w0 - 16) % 24
                    else:
                        src, src_col = t7, w0 - 112
                ps = ps_slab.tile([128, H], f32, tag="pslab")
                nc.tensor.matmul(ps[:], src[:, src_col:src_col + 2, :], ident[:],
                                 is_transpose=True, start=True, stop=True)
                sl = slab_pool.tile([128, H + 2], f32r, tag=tag)
                nc.vector.tensor_copy(sl[:, 1:H + 1], ps[:])
                nc.vector.tensor_copy(sl[:, 0:1], ps[:, H - 1:H])
                nc.vector.tensor_copy(sl[:, H + 1:H + 2], ps[:, 0:1])
                return sl

            for _rep in range(reps):
              for b in range(BPC):
                if b == 0 and _rep == 0:
                    E = {NE - 1: make_eslab(b, NE - 1, tag="e63",
                                            src=start_a, src_col=0),
                         0: make_eslab(b, 0, tag="e0", src=start_b, src_col=0),
                         1: make_eslab(b, 1, tag="slab", src=start_b, src_col=2)}
                else:
                    E = {NE - 1: make_eslab(b, NE - 1, tag="e63"),
                         0: make_eslab(b, 0, tag="e0"),
                         1: make_eslab(b, 1, tag="slab")}
                for w in range(W):
                    i = w // 2
                    po = ps_out.tile([H, F], f32, tag="pout")
                    if w % 2 == 0:
                        pair_sl, pair_k = E[i], kp1
                        sng = E[(i - 1) % NE]
                        sng_lo, sng_hi, sng_k = C, 2 * C, ks_hi
                    else:
                        pair_sl, pair_k = E[i], kp2
                        sng = E[(i + 1) % NE]
                        sng_lo, sng_hi, sng_k = 0, C, ks_lo
                    for dhi in range(3):
                        off = 2 - dhi  # dh = [-1,0,1][dhi] -> off = 1-dh
                        nc.tensor.matmul(
                            po[:], pair_sl[:, off:off + H], pair_k[dhi][:],
                            start=(dhi == 0), stop=False,
                        )
                        nc.tensor.matmul(
                            po[:], sng[sng_lo:sng_hi, off:off + H],
                            sng_k[dhi][sng_lo:sng_hi, :],
                            start=False, stop=(dhi == 2),
                        )
                    if w % 4 == 0:
                        ob = out_pool.tile([H, 4, F], f32, tag="outsb")
                        ob_quad = ob
                    else:
                        ob = ob_quad
                    nc.vector.tensor_add(ob[:, w % 4, :], po[:], biasf[:])
                    if w % 4 == 3:
                        eng = nc.sync if w % 8 == 3 else nc.scalar
                        eng.dma_start(out_d[b, :, w - 3:w + 1, :], ob[:])
                    # software pipeline: produce E[i+2] at even steps
                    if w % 2 == 0 and i + 2 <= NE - 2:
                        E[i + 2] = make_eslab(b, i + 2, tag="slab")
                    if w % 2 == 1 and i >= 2:
                        E.pop(i - 1, None)

    nc.compile()
    return nc


_NC_CACHE = {}


def _get_nc(reps=1):
    if reps not in _NC_CACHE:
        _NC_CACHE[reps] = _build_module(reps)
    return _NC_CACHE[reps]


def kernel(x, kernel, bias, _trace=False):
    from concourse.bass_utils import run_bass_kernel_spmd

    x = np.ascontiguousarray(np.asarray(x, dtype=np.float32))
    kern = np.ascontiguousarray(np.asarray(kernel, dtype=np.float32))
    bias = np.asarray(bias, dtype=np.float32)
    biasf = np.ascontiguousarray(np.broadcast_to(bias[None, :], (128, F)))
    ident = np.eye(128, dtype=np.float32)

    nc = _get_nc()
    in_maps = [
        {"xc": x[c * BPC:(c + 1) * BPC], "kw": kern, "biasf": biasf,
         "ident": ident}
        for c in range(NCORES)
    ]
    res = run_bass_kernel_spmd(nc, in_maps, core_ids=list(range(NCORES)),
                               trace=_trace)
    out = np.concatenate([res.results[c]["out"] for c in range(NCORES)], axis=0)
    if _trace:
        kernel._last_results = res
    return out
